# revision 1
# baseline (speedup 1.0000x reference)
"""BertSelfAttention (B=2, S=2048, H=1024, 16 heads x 64) on 8 TRN2 NeuronCores.

Sharding: data parallel on batch (4 cores per batch) x tensor parallel on
heads (4 heads per core). No cross-core comms; each core computes
out[b, :, 256*g:256*(g+1)] for its head group g.

v5: head-PAIR attention loop so the two K=64 scores matmuls land in
different PE row groups (rows 0-63 / 64-127) and run concurrently; one
[128,1024] exp per (pair, qb, k) covers both heads. Deep et buffering lets
ctx/v-proj PE work lag behind the ACT-bound exp stream without stalling it.
All matmuls bf16; hiddenT via serial X-bar DMA transposes on the sync queue
(concurrent transposes corrupt); weights load on the scalar queue meanwhile.

Per-core pipeline:
  A) hiddenT [128(j), 2048(s)] bf16 via 8 transposing DMAs (sync, serial)
  B) kT/qT [128(d of head pair), 2048(s)] bf16 (1/8 scale + bias folded),
     V [128(s), 4heads, 65] bf16 with ones column (denominator for free);
     all projection groups JIT-emitted into the attention stream
  C) per (pair, qb=512, k): scoresT h0|h1 packed -> one exp [128,1024]
     (mask bias) -> bf16 et, ctxT[65, 512] += v_ext.T @ et per head
  D) PE-transpose ctxT (bf16) -> [q, 65], DVE reciprocal + scale, DMA out
"""

import ml_dtypes
import numpy as np

import concourse.bass as bass
import concourse.tile as tile
from concourse import bacc, mybir
from concourse.bass_utils import run_bass_kernel_spmd
from concourse.masks import make_identity

F32 = mybir.dt.float32
BF16 = mybir.dt.bfloat16
EXP = mybir.ActivationFunctionType.Exp

B, S, H = 2, 2048, 1024
NH, HD = 16, 64
NCORES = 8
HPC = 4  # heads per core
DPC = HPC * HD  # 256 output dims per core
SC = S // 128  # 16 s/k chunks
JC = H // 128  # 8 contraction chunks
QB = 512  # q block in attention inner loop
NQB = S // QB  # 4


def build():
    nc = bacc.Bacc(
        "TRN2",
        target_bir_lowering=False,
        debug=False,
        enable_asserts=False,
        num_devices=NCORES,
    )
    hidb = nc.dram_tensor("hidb", [S, H], BF16, kind="ExternalInput").ap()
    wq = nc.dram_tensor("wq", [H, DPC], BF16, kind="ExternalInput").ap()
    wk = nc.dram_tensor("wk", [H, DPC], BF16, kind="ExternalInput").ap()
    wv = nc.dram_tensor("wv", [H, DPC], BF16, kind="ExternalInput").ap()
    bqs = nc.dram_tensor("bqs", [128, 2], F32, kind="ExternalInput").ap()
    bks = nc.dram_tensor("bks", [128, 2], F32, kind="ExternalInput").ap()
    bvs = nc.dram_tensor("bvs", [1, DPC], BF16, kind="ExternalInput").ap()
    mask = nc.dram_tensor("mask", [128, SC], F32, kind="ExternalInput").ap()
    out = nc.dram_tensor("out", [S, DPC], F32, kind="ExternalOutput").ap()

    with tile.TileContext(nc) as tc:
        with (
            tc.tile_pool(name="persist", bufs=1) as persist,
            tc.tile_pool(name="etp", bufs=14) as etp,
            tc.tile_pool(name="ctsp", bufs=2) as ctsp,
            tc.tile_pool(name="rcp", bufs=4) as rcp,
            tc.tile_pool(name="scps", bufs=2, space="PSUM") as scps,
            tc.tile_pool(name="ctxps", bufs=1, space="PSUM") as ctxps,
            tc.tile_pool(name="vdps", bufs=2, space="PSUM") as vdps,
        ):
            ones1_f = persist.tile([1, 128], F32, tag="ones1f")
            nc.vector.memset(ones1_f[:], 1.0)
            # warm the ACT exp table during startup
            warm = persist.tile([1, 1], F32, tag="warm")
            nc.scalar.activation(warm[:], ones1_f[:, 0:1], EXP)

            # constants + weights first on the sync queue (needed by the
            # first projection groups), then the transposes
            mask_sb = persist.tile([128, SC], F32, tag="mask")
            nc.sync.dma_start(mask_sb[:], mask)
            bqs_sb = persist.tile([128, 2], F32, tag="bqs")
            nc.sync.dma_start(bqs_sb[:], bqs)
            bks_sb = persist.tile([128, 2], F32, tag="bks")
            nc.sync.dma_start(bks_sb[:], bks)
            bvs_sb = persist.tile([1, DPC], BF16, tag="bvs")
            nc.sync.dma_start(bvs_sb[:], bvs)
            w_sb = {}
            for name, w in (("wk", wk), ("wq", wq), ("wv", wv)):
                t = persist.tile([128, JC, DPC], BF16, tag=name, name=f"w_{name}")
                nc.sync.dma_start(t[:], w.rearrange("(c p) n -> p c n", p=128))
                w_sb[name] = t

            ident = persist.tile([128, 128], F32, tag="ident")
            make_identity(nc, ident[:])
            ident_bf = persist.tile([65, 65], BF16, tag="identbf")
            nc.vector.tensor_copy(ident_bf[:], ident[0:65, 0:65])
            ones1 = persist.tile([1, 128], BF16, tag="ones1")
            nc.vector.tensor_copy(ones1[:], ones1_f[:])
            ones4_f = persist.tile([128, HPC], F32, tag="ones4f")
            nc.vector.memset(ones4_f[:], 1.0)

            # persistent activations
            qT = [
                persist.tile([128, S], BF16, tag=f"qT{p}", name=f"qT{p}")
                for p in range(2)
            ]
            kT = [
                persist.tile([128, S], BF16, tag=f"kT{p}", name=f"kT{p}")
                for p in range(2)
            ]
            v_sb = [
                persist.tile([128, HPC, 65], BF16, tag=f"v{s}", name=f"v{s}")
                for s in range(SC)
            ]
            out_sb = [
                persist.tile([128, DPC], F32, tag=f"o{s}", name=f"o{s}")
                for s in range(SC)
            ]
            # hiddenT: a fast first 512-row block (8 small transposes, so the
            # first projections can start early) + one big transpose per j
            # for the remaining 1536 rows. All on the sync queue, serial
            # (concurrent X-bar transposes corrupt data).
            hidT0 = [
                persist.tile([128, 512], BF16, tag=f"hT0_{j}", name=f"hT0_{j}")
                for j in range(JC)
            ]
            hidT1 = [
                persist.tile([128, 512], BF16, tag=f"hT1_{j}", name=f"hT1_{j}")
                for j in range(JC)
            ]
            hidTr = [
                persist.tile([128, 1024], BF16, tag=f"hTr_{j}", name=f"hTr_{j}")
                for j in range(JC)
            ]
            for j in range(JC):
                nc.sync.dma_start_transpose(
                    out=hidT0[j][:], in_=hidb[0:512, j * 128 : (j + 1) * 128]
                )
            for j in range(JC):
                nc.sync.dma_start_transpose(
                    out=hidT1[j][:], in_=hidb[512:1024, j * 128 : (j + 1) * 128]
                )
            for j in range(JC):
                nc.sync.dma_start_transpose(
                    out=hidTr[j][:], in_=hidb[1024:S, j * 128 : (j + 1) * 128]
                )

            def hid_g(g, j):
                # 512-wide column group g of hiddenT chunk j
                if g == 0:
                    return hidT0[j][:]
                if g == 1:
                    return hidT1[j][:]
                return hidTr[j][:, (g - 2) * 512 : (g - 1) * 512]

            def hid_s(s, j):
                # 128-wide column chunk s of hiddenT chunk j
                if s < 4:
                    return hidT0[j][:, s * 128 : (s + 1) * 128]
                if s < 8:
                    return hidT1[j][:, (s - 4) * 128 : (s - 3) * 128]
                return hidTr[j][:, (s - 8) * 128 : (s - 7) * 128]

            def qk_pieces(wname, dst, bias, p, g):
                # one 512-wide output group (1 PSUM bank) of the qT/kT proj,
                # split into two 4-chunk pieces for fine-grained scheduling
                box = {}

                def piece_a():
                    ps = vdps.tile([128, 512], F32, tag="vd", name="projps_t")
                    box["ps"] = ps
                    for j in range(4):
                        nc.tensor.matmul(
                            ps[:],
                            w_sb[wname][:, j, p * 128 : (p + 1) * 128],
                            hid_g(g, j),
                            start=(j == 0),
                            stop=False,
                        )

                def piece_b():
                    ps = box["ps"]
                    for j in range(4, JC):
                        nc.tensor.matmul(
                            ps[:],
                            w_sb[wname][:, j, p * 128 : (p + 1) * 128],
                            hid_g(g, j),
                            start=False,
                            stop=(j == JC - 1),
                        )
                    nc.vector.tensor_scalar_add(
                        dst[p][:, g * 512 : (g + 1) * 512], ps[:], bias[:, p : p + 1]
                    )

                return piece_a, piece_b

            def qk_proj(wname, dst, bias, p, g):
                a, b = qk_pieces(wname, dst, bias, p, g)
                a()
                b()

            def v_proj(s):
                ps = vdps.tile([128, DPC], F32, tag="vd", name="vps_t")
                for j in range(JC):
                    nc.tensor.matmul(
                        ps[:],
                        hid_s(s, j),
                        w_sb["wv"][:, j, :],
                        start=(j == 0),
                        stop=False,
                    )
                nc.tensor.matmul(ps[:], ones1[:], bvs_sb[:], start=False, stop=True)
                ps3 = ps.rearrange("p (h c) -> p h c", h=HPC)
                nc.vector.tensor_copy(v_sb[s][:, :, 0:HD], ps3[:])
                nc.vector.tensor_copy(
                    v_sb[s][:, :, HD : HD + 1],
                    ones4_f[:].rearrange("p (h o) -> p h o", o=1),
                )

            def attention(pair, pre_k=None):
                # software-pipelined emission: scores for iteration i+1 are
                # emitted (= prioritized) BEFORE exp/fillers/ctx of iteration
                # i, so the ACT-bound exp stream never waits on filler work
                h0, h1 = 2 * pair, 2 * pair + 1
                iters = [(qb, k) for qb in range(NQB) for k in range(SC)]
                sts = {}

                def emit_scores(qb, k):
                    st = scps.tile([128, 2 * QB], F32, tag="sc", name="sc_t")
                    qs = qb * QB
                    # adjacent emission, opposite row groups -> the PE runs
                    # these two K=64 matmuls concurrently
                    nc.tensor.matmul(
                        st[:, 0:QB],
                        kT[pair][0:64, k * 128 : (k + 1) * 128],
                        qT[pair][0:64, qs : qs + QB],
                        start=True,
                        stop=True,
                    )
                    nc.tensor.matmul(
                        st[:, QB : 2 * QB],
                        kT[pair][64:128, k * 128 : (k + 1) * 128],
                        qT[pair][64:128, qs : qs + QB],
                        start=True,
                        stop=True,
                    )
                    sts[(qb, k)] = st

                ctxs = {}
                emit_scores(*iters[0])
                for idx, (qb, k) in enumerate(iters):
                    if k == 0:
                        ctxs[qb] = (
                            ctxps.tile([65, QB], F32, tag="ctx0", name="ctx0"),
                            ctxps.tile([65, QB], F32, tag="ctx1", name="ctx1"),
                        )
                    if idx + 1 < len(iters):
                        emit_scores(*iters[idx + 1])
                    st = sts.pop((qb, k))
                    et = etp.tile([128, 2 * QB], BF16, tag="et", name="et_t")
                    nc.scalar.activation(
                        et[:], st[:], EXP, bias=mask_sb[:, k : k + 1], scale=1.0
                    )
                    if pre_k:
                        for fn in pre_k.get((qb, k), ()):
                            fn()
                    ctx0, ctx1 = ctxs[qb]
                    nc.tensor.matmul(
                        ctx0[:],
                        v_sb[k][:, h0, :],
                        et[:, 0:QB],
                        start=(k == 0),
                        stop=(k == SC - 1),
                    )
                    nc.tensor.matmul(
                        ctx1[:],
                        v_sb[k][:, h1, :],
                        et[:, QB : 2 * QB],
                        start=(k == 0),
                        stop=(k == SC - 1),
                    )
                    if k == SC - 1:
                        for h, ctx in ((h0, ctx0), (h1, ctx1)):
                            cts = ctsp.tile([65, QB], BF16, tag="cts", name="cts_t")
                            nc.vector.tensor_copy(cts[:], ctx[:])
                            for i in range(QB // 128):
                                tp = vdps.tile(
                                    [128, 65], BF16, tag="vd", name="dps_t"
                                )
                                nc.tensor.transpose(
                                    tp[:],
                                    cts[:, i * 128 : (i + 1) * 128],
                                    ident_bf[:],
                                )
                                rc = rcp.tile([128, 1], F32, tag="rc", name="rc_t")
                                nc.vector.reciprocal(rc[:], tp[:, HD : HD + 1])
                                qc = qb * (QB // 128) + i
                                nc.vector.tensor_scalar_mul(
                                    out_sb[qc][:, h * HD : (h + 1) * HD],
                                    tp[:, 0:HD],
                                    rc[:],
                                )

            # minimal critical-path projections, then everything else JIT
            qk_proj("wk", kT, bks_sb, 0, 0)
            qk_proj("wq", qT, bqs_sb, 0, 0)
            pk0 = {(0, k): [lambda s=k: v_proj(s)] for k in range(SC)}

            def place(pk, slots, wname, dst, bias, p, g):
                a, b = qk_pieces(wname, dst, bias, p, g)
                pk.setdefault(slots[0], []).append(a)
                pk.setdefault(slots[1], []).append(b)

            place(pk0, [(0, 1), (0, 2)], "wk", kT, bks_sb, 0, 1)
            place(pk0, [(0, 5), (0, 6)], "wk", kT, bks_sb, 0, 2)
            place(pk0, [(0, 9), (0, 10)], "wk", kT, bks_sb, 0, 3)
            place(pk0, [(0, 12), (0, 13)], "wq", qT, bqs_sb, 0, 1)
            place(pk0, [(1, 4), (1, 5)], "wq", qT, bqs_sb, 0, 2)
            place(pk0, [(1, 8), (1, 9)], "wq", qT, bqs_sb, 0, 3)
            # pair-1 projections spread through pair-0's later blocks
            place(pk0, [(2, 3), (2, 4)], "wk", kT, bks_sb, 1, 0)
            place(pk0, [(2, 7), (2, 8)], "wk", kT, bks_sb, 1, 1)
            place(pk0, [(2, 11), (2, 12)], "wk", kT, bks_sb, 1, 2)
            place(pk0, [(3, 3), (3, 4)], "wk", kT, bks_sb, 1, 3)
            place(pk0, [(3, 7), (3, 8)], "wq", qT, bqs_sb, 1, 0)
            place(pk0, [(3, 11), (3, 12)], "wq", qT, bqs_sb, 1, 1)
            attention(0, pk0)
            pk1 = {}
            place(pk1, [(0, 5), (0, 6)], "wq", qT, bqs_sb, 1, 2)
            place(pk1, [(1, 5), (1, 6)], "wq", qT, bqs_sb, 1, 3)
            attention(1, pk1)

            for s in range(SC):
                nc.sync.dma_start(out[s * 128 : (s + 1) * 128, :], out_sb[s][:])

    nc.compile()
    return nc


def make_in_maps(hidden_states, attention_mask, Wq, bq, Wk, bk, Wv, bv):
    hidden_states = np.asarray(hidden_states, dtype=np.float32)
    attention_mask = np.asarray(attention_mask, dtype=np.float32)
    Wq = np.asarray(Wq, dtype=np.float32)
    bq = np.asarray(bq, dtype=np.float32)
    Wk = np.asarray(Wk, dtype=np.float32)
    bk = np.asarray(bk, dtype=np.float32)
    Wv = np.asarray(Wv, dtype=np.float32)
    bv = np.asarray(bv, dtype=np.float32)
    bf = ml_dtypes.bfloat16

    in_maps = []
    for c in range(NCORES):
        b = c // 4
        g = c % 4
        rows = slice(g * DPC, (g + 1) * DPC)
        in_maps.append(
            {
                "hidb": np.ascontiguousarray(hidden_states[b]).astype(bf),
                "wq": np.ascontiguousarray((Wq[rows, :] * 0.125).T).astype(bf),
                "wk": np.ascontiguousarray(Wk[rows, :].T).astype(bf),
                "wv": np.ascontiguousarray(Wv[rows, :].T).astype(bf),
                "bqs": np.ascontiguousarray((bq[rows] * 0.125).reshape(2, 128).T),
                "bks": np.ascontiguousarray(bk[rows].reshape(2, 128).T),
                "bvs": np.ascontiguousarray(bv[rows].reshape(1, DPC)).astype(bf),
                "mask": np.ascontiguousarray(
                    attention_mask[b, 0, 0, :].reshape(SC, 128).T
                ),
            }
        )
    return in_maps


def gather(results):
    full = np.empty((B, S, H), dtype=np.float32)
    for c in range(NCORES):
        b = c // 4
        g = c % 4
        full[b, :, g * DPC : (g + 1) * DPC] = results[c]["out"]
    return full


_NC = None


def kernel(hidden_states, attention_mask, Wq, bq, Wk, bk, Wv, bv, **run_kwargs):
    global _NC
    if _NC is None:
        _NC = build()
    in_maps = make_in_maps(hidden_states, attention_mask, Wq, bq, Wk, bk, Wv, bv)
    res = run_bass_kernel_spmd(_NC, in_maps, core_ids=list(range(NCORES)), **run_kwargs)
    out = gather(res.results)
    if run_kwargs:
        kernel.last_result = res
    return out

